# revision 51
# baseline (speedup 1.0000x reference)
"""BatchHardTripletLoss on 8 TRN2 NeuronCores (Bass/Tile).

Contract: kernel(**inputs) takes the FULL inputs (h1,h2,h3: [2048,512] f32)
and returns the full output tuple (loss, mean_diff, good, bad, rms_norm)
matching reference semantics:

    batch = concat(h1, h2)            # [4096, 512]
    d2[i,j] = sq[i] + sq[j] - 2 * (batch @ batch.T)[i,j]
    d = sqrt(max(d2, 1e-14)); d = max(d, 1e-7)
    hp[i] = d[i, (i+2048) % 4096]                  # the single positive
    hn[i] = min_{j not in {i, partner}} d[i, j]    # hardest negative

Sharding: rows (anchors) split 512/core across 8 cores. Each core gets a
column-ROTATED copy of batch.T (rolled by -512*core) so the kernel is
fully SPMD-static: its own diagonal block is always column-tile 0 and the
positive-pair block is always column-tile 4, with the excluded column at
static in-tile offset 128*m + p for row-chunk m, partition p.

Mining happens on f[i,j] = g[i,j] - sq[j]/2 straight out of PSUM
(argmin of d2 = argmax of f); the -sq[j]/2 term is folded into the PSUM
accumulation itself via one extra K=128 matmul per tile, so the DVE does
nothing but max-reduces. The device outputs raw f values; the host
applies d2 = sq_i - 2f, the clamps, and sqrt.

Device loop structure: column-tile n OUTER so PE consumption rate
matches DMA delivery (each 1MB column block feeds 20 matmuls), with one
[128, 2048] 4-bank PSUM tile per n holding all four 128-row chunks, so
the max-reduces run as single wide DVE ops.
"""

import os
import sys

import numpy as np

if "/opt/trn_rl_repo" not in sys.path:
    sys.path.insert(0, "/opt/trn_rl_repo")

N = 2048
TN = 2 * N          # 4096 rows in the distance matrix
D = 512             # feature dim
NCORES = 8
RB = TN // NCORES   # 512 rows per core
MCH = RB // 128     # 4 row-chunks of 128 per core
NT = TN // 512      # 8 column tiles of 512
KT = D // 128       # 4 contraction tiles of 128
NEG_BIG = -1.0e30

MM_DTYPE = os.environ.get("BASS_MM_DTYPE", "f32r")

_CACHE = {}

# test.py introspection: exec time of the last hardware run (ns) when
# BASS_KERNEL_TRACE=1, else None.
last_exec_ns = None
last_profile_json = None


def _build_nc():
    import concourse.bacc as bacc
    import concourse.mybir as mybir
    from concourse.tile import TileContext

    f32 = mybir.dt.float32
    mm_dt = {
        "f32r": mybir.dt.float32r,
        "f32": mybir.dt.float32,
        "bf16": mybir.dt.bfloat16,
    }[MM_DTYPE]
    Alu = mybir.AluOpType
    Ax = mybir.AxisListType

    nc = bacc.Bacc("TRN2", target_bir_lowering=False, debug=False)

    bt = nc.declare_dram_parameter("bt", [D, TN], mm_dt, isOutput=False)
    nsq = nc.declare_dram_parameter("nsq", [1, TN], mm_dt, isOutput=False)
    one = nc.declare_dram_parameter("one", [128, 128], mm_dt, isOutput=False)
    out = nc.declare_dram_parameter("out", [RB, 2], f32, isOutput=True)

    with TileContext(nc) as tc:
        with (
            tc.tile_pool(name="persist", bufs=1) as pp,
            tc.tile_pool(name="psum", bufs=2, space="PSUM") as psp,
            tc.tile_pool(name="work", bufs=3) as wp,
            tc.tile_pool(name="small", bufs=4) as sp,
        ):
            # --- loads -------------------------------------------------
            # Tiny transfers first so they don't queue behind 8MB of btk.
            onest = pp.tile([128, 128], mm_dt, name="onest")
            nc.sync.dma_start(out=onest[:, :], in_=one[:, :])
            nsqt = pp.tile([1, TN], mm_dt, name="nsqt")
            nc.sync.dma_start(out=nsqt[:, :], in_=nsq[0:1, :])

            # PE warm-up: the HAM clock gate holds the PE at 1.2 GHz until
            # ~3.4us of sustained activity. The PE sits idle waiting for
            # the first btk chunk anyway, so burn that window on dummy
            # matmuls against the constant tile; real matmuls then start
            # at full clock.
            wps = psp.tile([128, 128], f32, name="wps", tag="ps")
            for _ in range(16):
                nc.tensor.matmul(wps[:, :], onest[:, :], onest[:, :],
                                 start=True, stop=True)

            # K=128 augment operands: a K=1 matmul in the stream halves the
            # PE rate for every gram matmul, so broadcast -sq/2 across all
            # 128 partitions and contract against (1/128)*ones instead.
            # Chunked so the n=0 augment isn't gated on the full 16KB row.
            nsqb = pp.tile([128, TN], mm_dt, name="nsqb")
            for c in range(NT):
                nc.gpsimd.partition_broadcast(
                    nsqb[:, 512 * c : 512 * (c + 1)],
                    nsqt[:, 512 * c : 512 * (c + 1)],
                )

            btk = [pp.tile([128, TN], mm_dt, name=f"btk{k}") for k in range(KT)]
            # Column-chunked so compute on early column tiles starts while
            # later chunks are still in flight; later chunks are wider for
            # better DMA burst efficiency.
            bounds = [0, 512, 1024, 2048, 3072, 4096]
            for lo, hi in zip(bounds, bounds[1:]):
                for k in range(KT):
                    nc.sync.dma_start(
                        out=btk[k][:, lo:hi],
                        in_=bt[128 * k : 128 * (k + 1), lo:hi],
                    )

            # --- main grid: n outer, all 4 row-chunks per PSUM quad ----
            # PSUM accumulates f = g - sq_j/2 directly: the 4 K-tiles of
            # the gram matmul plus one K=128 "augment" matmul adding
            # (ones/128).T @ broadcast(-sq/2) to fold the column term in.
            # packed result: [:, m, 0] = f at the positive pair,
            #                [:, m, 1] = max over excluded-negatives f
            # [:, :, 1] doubles as the running cross-tile max so the last
            # combine is a tiny [128,4] op instead of a post-loop reduce.
            fout = pp.tile([128, MCH, 2], f32, name="fout")
            W = 512 * MCH  # 2048: full quad width
            for n in range(NT):
                ps = psp.tile([128, W], f32, name="ps", tag="ps")
                for m in range(MCH):
                    for k in range(KT):
                        nc.tensor.matmul(
                            ps[:, 512 * m : 512 * (m + 1)],
                            btk[k][:, 128 * m : 128 * (m + 1)],
                            btk[k][:, 512 * n : 512 * (n + 1)],
                            start=(k == 0),
                            stop=False,
                        )
                    nc.tensor.matmul(
                        ps[:, 512 * m : 512 * (m + 1)],
                        onest[:, :],
                        nsqb[:, 512 * n : 512 * (n + 1)],
                        start=False,
                        stop=True,
                    )
                ps3 = ps.rearrange("p (m j) -> p m j", m=MCH)
                if n == 0 or n == NT // 2:
                    # excluded column at offset 128*m + p of each chunk;
                    # affine_select runs on GpSimd which can't read PSUM,
                    # so bounce the quad through SBUF on the idle ScalarE.
                    fs = wp.tile([128, W], f32, name="fs", tag="fs")
                    nc.vector.tensor_copy(fs[:, :], ps[:, :])
                    fs3 = fs.rearrange("p (m j) -> p m j", m=MCH)
                    fx = wp.tile([128, W], f32, name="fx", tag="fx")
                    fx3 = fx.rearrange("p (m j) -> p m j", m=MCH)
                    nc.gpsimd.affine_select(
                        out=fx3,
                        in_=fs3,
                        pattern=[[-128, MCH], [1, 512]],
                        compare_op=Alu.not_equal,
                        fill=NEG_BIG,
                        base=0,
                        channel_multiplier=-1,
                    )
                    if n == 0:
                        nc.vector.tensor_reduce(
                            out=fout[:, :, 1], in_=fx3, axis=Ax.X, op=Alu.max
                        )
                    else:
                        pm = sp.tile([128, MCH], f32, name="pm", tag="pm")
                        nc.vector.tensor_reduce(
                            out=pm[:, :], in_=fx3, axis=Ax.X, op=Alu.max
                        )
                        nc.vector.tensor_tensor(
                            fout[:, :, 1], fout[:, :, 1], pm[:, :], op=Alu.max
                        )
                    if n == NT // 2:
                        # extract the positive-pair value f[i, partner]
                        fpx = wp.tile([128, W], f32, name="fpx", tag="fx")
                        fpx3 = fpx.rearrange("p (m j) -> p m j", m=MCH)
                        nc.gpsimd.affine_select(
                            out=fpx3,
                            in_=fs3,
                            pattern=[[-128, MCH], [1, 512]],
                            compare_op=Alu.is_equal,
                            fill=NEG_BIG,
                            base=0,
                            channel_multiplier=-1,
                        )
                        nc.vector.tensor_reduce(
                            out=fout[:, :, 0], in_=fpx3, axis=Ax.X, op=Alu.max
                        )
                else:
                    pm = sp.tile([128, MCH], f32, name="pm", tag="pm")
                    nc.vector.tensor_reduce(
                        out=pm[:, :], in_=ps3, axis=Ax.X, op=Alu.max
                    )
                    nc.vector.tensor_tensor(
                        fout[:, :, 1], fout[:, :, 1], pm[:, :], op=Alu.max
                    )

            # host applies d2 = sq_i - 2 f and the sqrt/clamps to [512,2].
            nc.sync.dma_start(
                out=out.rearrange("(m p) c -> p m c", m=MCH), in_=fout[:, :, :]
            )

    nc.finalize()
    return nc


def _get_nc():
    if "nc" not in _CACHE:
        _CACHE["nc"] = _build_nc()
    return _CACHE["nc"]


def kernel(h1, h2, h3=None, **_unused):
    global last_exec_ns, last_profile_json
    from concourse.bass_utils import run_bass_kernel_spmd

    h1 = np.asarray(h1, dtype=np.float32)
    h2 = np.asarray(h2, dtype=np.float32)
    batch = np.concatenate([h1, h2], axis=0)               # [4096, 512]
    bt = np.ascontiguousarray(batch.T)                     # [512, 4096]
    sq = np.sum(batch * batch, axis=1, dtype=np.float32)   # [4096]

    ones = np.full((128, 128), 1.0 / 128.0, np.float32)
    in_maps = []
    for c in range(NCORES):
        r0 = RB * c
        in_maps.append(
            {
                "bt": np.roll(bt, -r0, axis=1),
                "nsq": (np.roll(sq, -r0) * np.float32(-0.5))[None, :],
                "one": ones,
            }
        )

    nc = _get_nc()
    trace = os.environ.get("BASS_KERNEL_TRACE", "0") == "1"
    res = run_bass_kernel_spmd(nc, in_maps, list(range(NCORES)), trace=trace)
    last_exec_ns = res.exec_time_ns
    last_profile_json = res.profile_json

    outs = [res.results[c]["out"] for c in range(NCORES)]
    fpart = np.concatenate([o[:, 0] for o in outs])        # [4096]
    fmax = np.concatenate([o[:, 1] for o in outs])
    hp = np.sqrt(np.maximum(sq - np.float32(2.0) * fpart, np.float32(1e-14)))
    hn = np.sqrt(np.maximum(sq - np.float32(2.0) * fmax, np.float32(1e-14)))

    diff = (hp - hn).astype(np.float32)
    tl = np.maximum(diff + np.float32(0.1), np.float32(0.0))
    rel = tl > np.float32(1e-5)
    good = np.int32(np.sum(tl < np.float32(1e-5)))
    bad = np.int32(TN - good)
    n_rel = max(int(np.sum(rel)), 1)
    mean_rel = np.float32(np.sum(np.where(rel, tl, np.float32(0.0))) / n_rel)
    mean_diff = np.float32(np.mean(diff))
    rms = np.float32(np.sqrt(np.mean(sq)))
    loss = mean_rel
    return (loss, mean_diff, good, bad, rms)


# revision 54
# speedup vs baseline: 1.0119x; 1.0119x over previous
"""BatchHardTripletLoss on 8 TRN2 NeuronCores (Bass/Tile).

Contract: kernel(**inputs) takes the FULL inputs (h1,h2,h3: [2048,512] f32)
and returns the full output tuple (loss, mean_diff, good, bad, rms_norm)
matching reference semantics:

    batch = concat(h1, h2)            # [4096, 512]
    d2[i,j] = sq[i] + sq[j] - 2 * (batch @ batch.T)[i,j]
    d = sqrt(max(d2, 1e-14)); d = max(d, 1e-7)
    hp[i] = d[i, (i+2048) % 4096]                  # the single positive
    hn[i] = min_{j not in {i, partner}} d[i, j]    # hardest negative

Sharding: rows (anchors) split 512/core across 8 cores. Each core gets a
column-ROTATED copy of batch.T (rolled by -512*core) so the kernel is
fully SPMD-static: its own diagonal block is always column-tile 0 and the
positive-pair block is always column-tile 4, with the excluded column at
static in-tile offset 128*m + p for row-chunk m, partition p.

Mining happens on f[i,j] = g[i,j] - sq[j]/2 straight out of PSUM
(argmin of d2 = argmax of f); the -sq[j]/2 term is folded into the PSUM
accumulation itself via one extra K=128 matmul per tile, so the DVE does
nothing but max-reduces. The device outputs raw f values; the host
applies d2 = sq_i - 2f, the clamps, and sqrt.

Device loop structure: column-tile n OUTER so PE consumption rate
matches DMA delivery (each 1MB column block feeds 20 matmuls), with one
[128, 2048] 4-bank PSUM tile per n holding all four 128-row chunks, so
the max-reduces run as single wide DVE ops.
"""

import os
import sys

import numpy as np

if "/opt/trn_rl_repo" not in sys.path:
    sys.path.insert(0, "/opt/trn_rl_repo")

N = 2048
TN = 2 * N          # 4096 rows in the distance matrix
D = 512             # feature dim
NCORES = 8
RB = TN // NCORES   # 512 rows per core
MCH = RB // 128     # 4 row-chunks of 128 per core
NT = TN // 512      # 8 column tiles of 512
KT = D // 128       # 4 contraction tiles of 128
NEG_BIG = -1.0e30

MM_DTYPE = os.environ.get("BASS_MM_DTYPE", "f32r")

_CACHE = {}

# test.py introspection: exec time of the last hardware run (ns) when
# BASS_KERNEL_TRACE=1, else None.
last_exec_ns = None
last_profile_json = None


def _build_nc():
    import concourse.bacc as bacc
    import concourse.mybir as mybir
    from concourse.tile import TileContext

    f32 = mybir.dt.float32
    mm_dt = {
        "f32r": mybir.dt.float32r,
        "f32": mybir.dt.float32,
        "bf16": mybir.dt.bfloat16,
    }[MM_DTYPE]
    Alu = mybir.AluOpType
    Ax = mybir.AxisListType

    nc = bacc.Bacc("TRN2", target_bir_lowering=False, debug=False)

    bt = nc.declare_dram_parameter("bt", [D, TN], mm_dt, isOutput=False)
    nsq = nc.declare_dram_parameter("nsq", [1, TN], mm_dt, isOutput=False)
    one = nc.declare_dram_parameter("one", [128, 128], mm_dt, isOutput=False)
    out = nc.declare_dram_parameter("out", [RB, 2], f32, isOutput=True)

    with TileContext(nc) as tc:
        with (
            tc.tile_pool(name="persist", bufs=1) as pp,
            tc.tile_pool(name="psum", bufs=2, space="PSUM") as psp,
            tc.tile_pool(name="work", bufs=4) as wp,
            tc.tile_pool(name="small", bufs=8) as sp,
        ):
            # --- loads -------------------------------------------------
            # Tiny transfers first so they don't queue behind 8MB of btk.
            onest = pp.tile([128, 128], mm_dt, name="onest")
            nc.sync.dma_start(out=onest[:, :], in_=one[:, :])
            nsqt = pp.tile([1, TN], mm_dt, name="nsqt")
            nc.sync.dma_start(out=nsqt[:, :], in_=nsq[0:1, :])

            # PE warm-up: the HAM clock gate holds the PE at 1.2 GHz until
            # ~3.4us of sustained activity. The PE sits idle waiting for
            # the first btk chunk anyway, so burn that window on dummy
            # matmuls against the constant tile; real matmuls then start
            # at full clock.
            wps = psp.tile([128, 128], f32, name="wps", tag="ps")
            for _ in range(16):
                nc.tensor.matmul(wps[:, :], onest[:, :], onest[:, :],
                                 start=True, stop=True)

            # K=128 augment operands: a K=1 matmul in the stream halves the
            # PE rate for every gram matmul, so broadcast -sq/2 across all
            # 128 partitions and contract against (1/128)*ones instead.
            # Chunked so the n=0 augment isn't gated on the full 16KB row.
            nsqb = pp.tile([128, TN], mm_dt, name="nsqb")
            for c in range(NT):
                nc.gpsimd.partition_broadcast(
                    nsqb[:, 512 * c : 512 * (c + 1)],
                    nsqt[:, 512 * c : 512 * (c + 1)],
                )

            btk = [pp.tile([128, TN], mm_dt, name=f"btk{k}") for k in range(KT)]
            # Column-chunked so compute on early column tiles starts while
            # later chunks are still in flight; later chunks are wider for
            # better DMA burst efficiency.
            bounds = [0, 512, 1024, 2048, 3072, 4096]
            for lo, hi in zip(bounds, bounds[1:]):
                for k in range(KT):
                    nc.sync.dma_start(
                        out=btk[k][:, lo:hi],
                        in_=bt[128 * k : 128 * (k + 1), lo:hi],
                    )

            # --- main grid: n outer, all 4 row-chunks per PSUM quad ----
            # PSUM accumulates f = g - sq_j/2 directly: the 4 K-tiles of
            # the gram matmul plus one K=128 "augment" matmul adding
            # (ones/128).T @ broadcast(-sq/2) to fold the column term in.
            # packed result: [:, m, 0] = f at the positive pair,
            #                [:, m, 1] = max over excluded-negatives f
            # [:, :, 1] doubles as the running cross-tile max so the last
            # combine is a tiny [128,4] op instead of a post-loop reduce.
            fout = pp.tile([128, MCH, 2], f32, name="fout")
            W = 512 * MCH  # 2048: full quad width
            for n in range(NT):
                ps = psp.tile([128, W], f32, name="ps", tag="ps")
                for m in range(MCH):
                    for k in range(KT):
                        nc.tensor.matmul(
                            ps[:, 512 * m : 512 * (m + 1)],
                            btk[k][:, 128 * m : 128 * (m + 1)],
                            btk[k][:, 512 * n : 512 * (n + 1)],
                            start=(k == 0),
                            stop=False,
                        )
                    nc.tensor.matmul(
                        ps[:, 512 * m : 512 * (m + 1)],
                        onest[:, :],
                        nsqb[:, 512 * n : 512 * (n + 1)],
                        start=False,
                        stop=True,
                    )
                ps3 = ps.rearrange("p (m j) -> p m j", m=MCH)
                if n == 0 or n == NT // 2:
                    # excluded column at offset 128*m + p of each chunk;
                    # affine_select runs on GpSimd which can't read PSUM,
                    # so bounce the quad through SBUF on the idle ScalarE.
                    fs = wp.tile([128, W], f32, name="fs", tag="fs")
                    nc.vector.tensor_copy(fs[:, :], ps[:, :])
                    fs3 = fs.rearrange("p (m j) -> p m j", m=MCH)
                    fx = wp.tile([128, W], f32, name="fx", tag="fx")
                    fx3 = fx.rearrange("p (m j) -> p m j", m=MCH)
                    nc.gpsimd.affine_select(
                        out=fx3,
                        in_=fs3,
                        pattern=[[-128, MCH], [1, 512]],
                        compare_op=Alu.not_equal,
                        fill=NEG_BIG,
                        base=0,
                        channel_multiplier=-1,
                    )
                    if n == 0:
                        nc.vector.tensor_reduce(
                            out=fout[:, :, 1], in_=fx3, axis=Ax.X, op=Alu.max
                        )
                    else:
                        pm = sp.tile([128, MCH], f32, name="pm", tag="pm")
                        nc.vector.tensor_reduce(
                            out=pm[:, :], in_=fx3, axis=Ax.X, op=Alu.max
                        )
                        nc.vector.tensor_tensor(
                            fout[:, :, 1], fout[:, :, 1], pm[:, :], op=Alu.max
                        )
                    if n == NT // 2:
                        # extract the positive-pair value f[i, partner]
                        fpx = wp.tile([128, W], f32, name="fpx", tag="fx")
                        fpx3 = fpx.rearrange("p (m j) -> p m j", m=MCH)
                        nc.gpsimd.affine_select(
                            out=fpx3,
                            in_=fs3,
                            pattern=[[-128, MCH], [1, 512]],
                            compare_op=Alu.is_equal,
                            fill=NEG_BIG,
                            base=0,
                            channel_multiplier=-1,
                        )
                        nc.vector.tensor_reduce(
                            out=fout[:, :, 0], in_=fpx3, axis=Ax.X, op=Alu.max
                        )
                else:
                    pm = sp.tile([128, MCH], f32, name="pm", tag="pm")
                    nc.vector.tensor_reduce(
                        out=pm[:, :], in_=ps3, axis=Ax.X, op=Alu.max
                    )
                    nc.vector.tensor_tensor(
                        fout[:, :, 1], fout[:, :, 1], pm[:, :], op=Alu.max
                    )

            # host applies d2 = sq_i - 2 f and the sqrt/clamps to [512,2].
            nc.sync.dma_start(
                out=out.rearrange("(m p) c -> p m c", m=MCH), in_=fout[:, :, :]
            )

    nc.finalize()
    return nc


def _get_nc():
    if "nc" not in _CACHE:
        _CACHE["nc"] = _build_nc()
    return _CACHE["nc"]


def kernel(h1, h2, h3=None, **_unused):
    global last_exec_ns, last_profile_json
    from concourse.bass_utils import run_bass_kernel_spmd

    h1 = np.asarray(h1, dtype=np.float32)
    h2 = np.asarray(h2, dtype=np.float32)
    batch = np.concatenate([h1, h2], axis=0)               # [4096, 512]
    bt = np.ascontiguousarray(batch.T)                     # [512, 4096]
    sq = np.sum(batch * batch, axis=1, dtype=np.float32)   # [4096]

    ones = np.full((128, 128), 1.0 / 128.0, np.float32)
    in_maps = []
    for c in range(NCORES):
        r0 = RB * c
        in_maps.append(
            {
                "bt": np.roll(bt, -r0, axis=1),
                "nsq": (np.roll(sq, -r0) * np.float32(-0.5))[None, :],
                "one": ones,
            }
        )

    nc = _get_nc()
    trace = os.environ.get("BASS_KERNEL_TRACE", "0") == "1"
    res = run_bass_kernel_spmd(nc, in_maps, list(range(NCORES)), trace=trace)
    last_exec_ns = res.exec_time_ns
    last_profile_json = res.profile_json

    outs = [res.results[c]["out"] for c in range(NCORES)]
    fpart = np.concatenate([o[:, 0] for o in outs])        # [4096]
    fmax = np.concatenate([o[:, 1] for o in outs])
    hp = np.sqrt(np.maximum(sq - np.float32(2.0) * fpart, np.float32(1e-14)))
    hn = np.sqrt(np.maximum(sq - np.float32(2.0) * fmax, np.float32(1e-14)))

    diff = (hp - hn).astype(np.float32)
    tl = np.maximum(diff + np.float32(0.1), np.float32(0.0))
    rel = tl > np.float32(1e-5)
    good = np.int32(np.sum(tl < np.float32(1e-5)))
    bad = np.int32(TN - good)
    n_rel = max(int(np.sum(rel)), 1)
    mean_rel = np.float32(np.sum(np.where(rel, tl, np.float32(0.0))) / n_rel)
    mean_diff = np.float32(np.mean(diff))
    rms = np.float32(np.sqrt(np.mean(sq)))
    loss = mean_rel
    return (loss, mean_diff, good, bad, rms)
